# revision 1
# baseline (speedup 1.0000x reference)
"""Trainium2 Bass kernel for ComplexAttention.

Math (per (b,t) pair):
    cur2 = [cur_r, cur_i]                       # [2D]
    Q    = cur2 @ qW + qb                       # [D]
    K_s  = H_s @ kW + kb ; V_s = H_s @ vW + vb  # H = [hist_r, hist_i]  [S, 2D]
    sc_s = (Q . K_s) * scale * conf
    w    = softmax(sc) ; ctx = sum_s w_s V_s
    out  = cur + 0.1 * ctx (complex)

Rewrites used (exact):
    Q . K_s = (cur2 @ (qW kW^T) + qb kW^T) . H_s  +  (Q . kb)
        - the (Q . kb) term is constant over s -> softmax-invariant -> dropped.
    ctx = (sum_s w_s H_s) @ vW + vb      (since sum_s w_s = 1)

So per pair only two small contractions touch H (scores dot and the weighted
sum), and the heavy lifting is three big batched matmuls folded into two:
    Qk  = cur2 @ Wqk + bqk      (Wqk = qW kW^T, bqk = qb kW^T, host-folded)
    ctx = hbar @ vW + vb

Sharding: data-parallel over the 4096 (b,t) pairs, 512 per core, weights
replicated.  Pairs are processed in groups of 4 (stacked 4x32 on the 128 SBUF
partitions), 8 groups per H DMA sub-batch, 32 groups per 128-pair batch.
"""

import os
import sys

import numpy as np

os.environ.setdefault("MYCRO_LOCAL_CACHE", "1")

try:
    import concourse.bass as bass
except ImportError:  # pragma: no cover
    sys.path.insert(0, "/opt/trn_rl_repo")
    import concourse.bass as bass

import concourse.mybir as mybir
import concourse.tile as tile
from concourse import bacc
from concourse.bass_utils import run_bass_kernel_spmd

F32 = mybir.dt.float32
F32R = mybir.dt.float32r
F16 = mybir.dt.float16
AX = mybir.AluOpType
AF = mybir.ActivationFunctionType

B, T, S, D = 4, 1024, 32, 512
D2 = 2 * D  # 1024, concat(real, imag) feature dim
E = 2 * D   # 1024, history feature dim
N_CORES = 8
PAIRS = B * T
SCALE = float(D) ** -0.5


def r(ap):
    return ap.bitcast(F32R)


DEBUG_STOP = os.environ.get("K_DEBUG_STOP", "")  # "", "A", "B", "NOHB", "NOCTX"
BENCH_LOOP = int(os.environ.get("K_BENCH_LOOP", "0"))  # >0: repeat body N times
NO_TTR = os.environ.get("K_NO_TTR", "") == "1"   # replace fused dot with memset
NO_HDMA = os.environ.get("K_NO_HDMA", "") == "1"  # memset H instead of DMA load


def build(ppc: int) -> bass.Bass:
    """Build the per-core SPMD program for `ppc` pairs per core."""
    assert ppc % 128 == 0
    nb = ppc // 128      # batches of 128 pairs
    ng = ppc // 4        # groups of 4 pairs (core total)

    nc = bacc.Bacc("TRN2", target_bir_lowering=False)

    hist_r = nc.declare_dram_parameter("hist_real", [ppc, S, D], F32, isOutput=False)
    hist_i = nc.declare_dram_parameter("hist_imag", [ppc, S, D], F32, isOutput=False)
    cur_r = nc.declare_dram_parameter("cur_r", [ppc, D], F32, isOutput=False)
    cur_i = nc.declare_dram_parameter("cur_i", [ppc, D], F32, isOutput=False)
    cur2t = nc.declare_dram_parameter("cur2t", [D2, ppc], F32, isOutput=False)
    wqk = nc.declare_dram_parameter("wqk", [D2, E], F32, isOutput=False)
    bqk = nc.declare_dram_parameter("bqk", [1, E], F32, isOutput=False)
    vw = nc.declare_dram_parameter("vw", [E, E], F16, isOutput=False)
    vb = nc.declare_dram_parameter("vb", [1, E], F16, isOutput=False)
    conf_rep = nc.declare_dram_parameter("conf_rep", [128, ng], F32, isOutput=False)
    r32 = nc.declare_dram_parameter("r32", [128, 8, 128], F32, isOutput=False)
    m4 = nc.declare_dram_parameter("m4", [128, 4], F32, isOutput=False)
    ones1 = nc.declare_dram_parameter("ones1", [1, 128], F32, isOutput=False)
    ident = nc.declare_dram_parameter("ident", [128, 128], F16, isOutput=False)
    ones_h = nc.declare_dram_parameter("ones_h", [1, 128], F16, isOutput=False)
    out = nc.declare_dram_parameter("out", [ppc, D, 2], F32, isOutput=True)

    from contextlib import ExitStack

    with tile.TileContext(nc) as tc, ExitStack() as es:
            ec = es.enter_context
            cpool = ec(tc.tile_pool(name="const", bufs=1))
            wpool = ec(tc.tile_pool(name="bigw", bufs=1))
            curpool = ec(tc.tile_pool(name="cur", bufs=2))
            qkpool = ec(tc.tile_pool(name="qk", bufs=2))
            qk4pool = ec(tc.tile_pool(name="qk4", bufs=2))
            hpool = ec(tc.tile_pool(name="h", bufs=5))
            prodpool = ec(tc.tile_pool(name="prod", bufs=1))
            smpool = ec(tc.tile_pool(name="sm", bufs=4))
            wdpool = ec(tc.tile_pool(name="wd", bufs=4))
            hsbpool = ec(tc.tile_pool(name="hsb", bufs=4))
            hbpool = ec(tc.tile_pool(name="hbarb", bufs=2))
            htpool = ec(tc.tile_pool(name="hbarT", bufs=8))
            ctxpool = ec(tc.tile_pool(name="ctxs", bufs=2))
            outpool = ec(tc.tile_pool(name="outp", bufs=2))
            ps_sh = ec(tc.tile_pool(name="ps_sh", bufs=2, space="PSUM"))
            ps_rep = ec(tc.tile_pool(name="ps_rep", bufs=2, space="PSUM"))
            ps_hb = ec(tc.tile_pool(name="ps_hb", bufs=1, space="PSUM"))
            del es
            from contextlib import nullcontext
            loop_cm = (
                tc.For_i(0, BENCH_LOOP, 1) if BENCH_LOOP > 0 else nullcontext()
            )
            with loop_cm:
                # ---- constants / weights resident in SBUF ----
                m4_t = cpool.tile([128, 4], F32)
                nc.sync.dma_start(out=m4_t[:], in_=m4[:])
                ones_t = cpool.tile([1, 128], F32R)
                nc.sync.dma_start(out=ones_t[:], in_=ones1[:].bitcast(F32R))
                id_t = cpool.tile([128, 128], F16)
                nc.sync.dma_start(out=id_t[:], in_=ident[:])
                onesh_t = cpool.tile([1, 128], F16)
                nc.sync.dma_start(out=onesh_t[:], in_=ones_h[:])
                bqk_t = cpool.tile([1, E], F32R)
                nc.sync.dma_start(out=bqk_t[:], in_=bqk[:].bitcast(F32R))
                vb_t = cpool.tile([1, E], F16)
                nc.sync.dma_start(out=vb_t[:], in_=vb[:])
                cr_t = cpool.tile([128, ng], F32)
                nc.sync.dma_start(out=cr_t[:], in_=conf_rep[:])
                r32_t = cpool.tile([128, 8, 128], F32R)
                nc.sync.dma_start(out=r32_t[:], in_=r32[:].bitcast(F32R))

                # shares the H-pool slots: freed after phase A so H streaming
                # gets its second buffer back
                c2t_t = hpool.tile([128, 4, 2, ppc], F32R, tag="h")
                nc.sync.dma_start(
                    out=c2t_t[:],
                    in_=cur2t[:].bitcast(F32R).rearrange(
                        "(ka kb p) n -> p ka kb n", p=128, kb=2
                    ),
                )

                wqk_t = wpool.tile([128, 8, E], F32R, tag="bigw")
                nc.sync.dma_start(
                    out=wqk_t[:],
                    in_=wqk[:].bitcast(F32R).rearrange("(k p) e -> p k e", p=128),
                )

                # ---- phase A: Qk = cur2 @ Wqk + bqk, then relayout to [32,4,E]
                # so the per-group replication matmul always reads base partition 0
                qk4s = []
                for b in range(nb):
                    qk_t = qkpool.tile([128, E], F32, tag="qk")
                    for h in range(2):
                        ps = ps_sh.tile([128, 512], F32, tag="mm512")
                        for k in range(8):
                            nc.tensor.matmul(
                                ps[:],
                                lhsT=(
                                    c2t_t[:, k // 2, k % 2, 128 * b : 128 * (b + 1)]
                                ),
                                rhs=(wqk_t[:, k, 512 * h : 512 * (h + 1)]),
                                start=(k == 0),
                                stop=False,
                            )
                        nc.tensor.matmul(
                            ps[:],
                            lhsT=(ones_t[:]),
                            rhs=(bqk_t[:, 512 * h : 512 * (h + 1)]),
                            start=False,
                            stop=True,
                        )
                        nc.scalar.activation(
                            qk_t[:, 512 * h : 512 * (h + 1)], ps[:], AF.Copy
                        )
                    qk4_t = qk4pool.tile([32, 4, E], F32R, tag="qk4")
                    for blk in range(4):
                        nc.gpsimd.dma_start(
                            out=qk4_t[:, blk, :],
                            in_=qk_t[32 * blk : 32 * (blk + 1), :].bitcast(F32R),
                        )
                    qk4s.append(qk4_t)
                    if DEBUG_STOP == "A":
                        nc.sync.dma_start(
                            out=out[:]
                            .rearrange("(bb p) d two -> p bb (d two)", p=128)[:, b],
                            in_=qk_t[:],
                        )

                # vW reuses the Wqk SBUF slot once phase A has consumed it
                vw_t = wpool.tile([128, 8, E], F16, tag="bigw")
                nc.sync.dma_start(
                    out=vw_t[:],
                    in_=vw[:].rearrange("(k p) e -> p k e", p=128),
                )

                # ---- phases B (scores/softmax/hbar) + C (ctx/out) per batch ----
                for b in range(nb if DEBUG_STOP != "A" else 0):
                    cur_t = curpool.tile([128, 2, D], F32, tag="cur")
                    nc.sync.dma_start(
                        out=cur_t[:, 0, :],
                        in_=cur_r[128 * b : 128 * (b + 1), :],
                    )
                    nc.sync.dma_start(
                        out=cur_t[:, 1, :],
                        in_=cur_i[128 * b : 128 * (b + 1), :],
                    )
                    hbar_b = hbpool.tile([128, E], F16)
                    for sb in range(4):  # sub-batches of 8 groups = 32 pairs
                        hts4 = []
                        for half in range(2):
                            h_t = hpool.tile([128, 4, E], F32R, tag="h")
                            p0 = 4 * (32 * b + 8 * sb + 4 * half)
                            nc.sync.dma_start(
                                out=h_t[:, :, 0:D],
                                in_=hist_r[p0 : p0 + 16].bitcast(F32R).rearrange(
                                    "(gl j) s d -> (j s) gl d", j=4
                                ),
                            )
                            nc.sync.dma_start(
                                out=h_t[:, :, D:E],
                                in_=hist_i[p0 : p0 + 16].bitcast(F32R).rearrange(
                                    "(gl j) s d -> (j s) gl d", j=4
                                ),
                            )
                            hts4.append(h_t)

                        scores8 = smpool.tile([128, 8], F32, tag="scores")
                        exp8 = smpool.tile([128, 8], F32, tag="exp")
                        for gl in range(8):
                            g = 32 * b + 8 * sb + gl  # core-local group id
                            qkr = ps_rep.tile([128, E], F32)
                            for h in range(2):
                                nc.tensor.matmul(
                                    qkr[:, 512 * h : 512 * (h + 1)],
                                    lhsT=(r32_t[0:32, gl, :]),
                                    rhs=(qk4s[b][:, sb, 512 * h : 512 * (h + 1)]),
                                    start=True,
                                    stop=True,
                                )
                            # scores: DVE elementwise H*Qk_rep, then ACT
                            # Copy-with-accum folds the conf*scale and reduces
                            # along the free axis (the custom DVE fused reduce op
                            # hangs on this runtime, so standard ops only)
                            prod = prodpool.tile([128, E], F16)
                            nc.vector.tensor_tensor(
                                out=prod[:],
                                in0=hts4[gl // 4][:, gl % 4, :].bitcast(F32),
                                in1=qkr[:],
                                op=AX.mult,
                            )
                            sink = prodpool.tile([128, E], F16, tag="sink")
                            nc.vector.tensor_scalar(
                                sink[:],
                                prod[:],
                                cr_t[:, g : g + 1],
                                None,
                                AX.mult,
                                op1=AX.add,
                                accum_out=scores8[:, gl : gl + 1],
                            )
                        nc.scalar.activation(exp8[:], scores8[:], AF.Exp)

                        dn = ps_sh.tile([4, 8], F32, tag="mm512")
                        nc.tensor.matmul(dn[:], lhsT=m4_t[:], rhs=exp8[:], start=True, stop=True)
                        inv4 = smpool.tile([4, 8], F32, tag="inv")
                        nc.vector.reciprocal(inv4[:], dn[:])

                        if DEBUG_STOP == "B":
                            nc.sync.dma_start(
                                out=out[:]
                                .rearrange("(bb p) d two -> p bb (d two)", p=128)
                                [:, b, 8 * sb : 8 * (sb + 1)],
                                in_=exp8[:],
                            )
                        for gl in range(8 if DEBUG_STOP != "B" else 0):
                            wd = wdpool.tile([128, 4], F32R)
                            nc.vector.tensor_scalar_mul(wd[:], m4_t[:], exp8[:, gl : gl + 1])
                            hb4 = ps_hb.tile([4, E], F32)
                            for h in range(2):
                                nc.tensor.matmul(
                                    hb4[:, 512 * h : 512 * (h + 1)],
                                    lhsT=(wd[:]),
                                    rhs=(
                                        hts4[gl // 4][:, gl % 4, 512 * h : 512 * (h + 1)]
                                    ),
                                    start=True,
                                    stop=True,
                                )
                            hsb4 = hsbpool.tile([4, E], F16)
                            nc.scalar.activation(
                                hsb4[:], hb4[:], AF.Copy, scale=inv4[:, gl : gl + 1]
                            )
                            lp = 4 * (8 * sb + gl)  # batch-local pair of group
                            nc.gpsimd.dma_start(
                                out=hbar_b[lp : lp + 4, :], in_=hsb4[:]
                            )

                    if DEBUG_STOP == "NOCTX":
                        nc.sync.dma_start(
                            out=out[:]
                            .rearrange("(bb p) d two -> p bb (d two)", p=128)[:, b],
                            in_=hbar_b[:],
                        )
                    # transpose hbar [128 pairs, E] -> hbarT chunks [128 e, 128 p]
                    hts = []
                    for c in range(8 if DEBUG_STOP not in ("B", "NOCTX") else 0):
                        tp = ps_sh.tile([128, 128], F16, tag="mm512")
                        nc.tensor.transpose(
                            tp[:], hbar_b[:, 128 * c : 128 * (c + 1)], id_t[:]
                        )
                        ht = htpool.tile([128, 128], F16, tag="hbarT")
                        nc.scalar.activation(ht[:], tp[:], AF.Copy)
                        hts.append(ht)

                    out_t = outpool.tile([128, D, 2], F32)
                    for h2 in range(2 if DEBUG_STOP not in ("B", "NOCTX") else 0):
                        cps = ps_sh.tile([128, 512], F32, tag="mm512")
                        for c in range(8):
                            nc.tensor.matmul(
                                cps[:],
                                lhsT=(hts[c][:]),
                                rhs=(vw_t[:, c, 512 * h2 : 512 * (h2 + 1)]),
                                start=(c == 0),
                                stop=False,
                            )
                        nc.tensor.matmul(
                            cps[:],
                            lhsT=(onesh_t[:]),
                            rhs=(vb_t[:, 512 * h2 : 512 * (h2 + 1)]),
                            start=False,
                            stop=True,
                        )
                        nc.vector.scalar_tensor_tensor(
                            out=out_t[:, :, h2],
                            in0=cps[:],
                            scalar=0.1,
                            in1=cur_t[:, h2, :],
                            op0=AX.mult,
                            op1=AX.add,
                        )
                    if DEBUG_STOP not in ("B", "NOCTX"):
                        nc.sync.dma_start(
                            out=out[:]
                            .rearrange("(b p) d two -> p b d two", p=128)[:, b],
                            in_=out_t[:],
                        )

    # bacc lowering: splits multi-wait instructions (walrus allows only one
    # sync wait per instruction), register allocation, DCE
    nc.compile()
    return nc


_CACHE: dict[int, bass.Bass] = {}


def get_nc(ppc: int) -> bass.Bass:
    if ppc not in _CACHE:
        _CACHE[ppc] = build(ppc)
    return _CACHE[ppc]


def make_const_inputs(ng: int):
    r32_v = np.zeros((8, 32, 128), np.float32)
    for v in range(8):
        for j in range(4):
            r32_v[v, 4 * v + j, 32 * j : 32 * (j + 1)] = 1.0
    # replicated into each 32-partition block so lhsT base can match rhs base
    r32_h = np.ascontiguousarray(np.tile(r32_v.transpose(1, 0, 2), (4, 1, 1)))
    m4_h = np.zeros((128, 4), np.float32)
    for j in range(4):
        m4_h[32 * j : 32 * (j + 1), j] = 1.0
    ones_h = np.ones((1, 128), np.float32)
    id_h = np.eye(128, dtype=np.float16)
    return r32_h, m4_h, ones_h, id_h


def host_prep(hist_real, hist_imag, current_real, current_imag, confidence,
              qW, qb, kW, kb, vW, vb, ppc):
    """Shared host-side folding + per-core input maps."""
    f = lambda x: np.ascontiguousarray(np.asarray(x, dtype=np.float32))
    hist_real, hist_imag = f(hist_real), f(hist_imag)
    current_real, current_imag = f(current_real), f(current_imag)
    confidence = f(confidence)
    qW, qb, kW, kb, vW, vb = f(qW), f(qb), f(kW), f(kb), f(vW), f(vb)

    n_cores = (B * T) // ppc
    wqk_h = np.ascontiguousarray(qW @ kW.T)          # [D2, E]
    bqk_h = (qb @ kW.T).reshape(1, E)                # [1, E]
    vw_h = vW.astype(np.float16)
    vb_h = vb.reshape(1, E).astype(np.float16)
    ng = ppc // 4
    r32_h, m4_h, ones_h, id_h = make_const_inputs(ng)

    hr = hist_real.reshape(B * T, S, D)
    hi = hist_imag.reshape(B * T, S, D)
    cr = current_real.reshape(B * T, D)
    ci = current_imag.reshape(B * T, D)
    cf = confidence.reshape(B * T)

    in_maps = []
    for c in range(n_cores):
        sl = slice(c * ppc, (c + 1) * ppc)
        cur2t_h = np.ascontiguousarray(
            np.concatenate([cr[sl], ci[sl]], axis=1).T
        )  # [D2, ppc]
        c4 = cf[sl].reshape(ng, 4).T * SCALE          # [4, ng]
        conf_rep_h = np.ascontiguousarray(np.repeat(c4, 32, axis=0))  # [128, ng]
        in_maps.append({
            "hist_real": hr[sl],
            "hist_imag": hi[sl],
            "cur_r": cr[sl],
            "cur_i": ci[sl],
            "cur2t": cur2t_h,
            "wqk": wqk_h,
            "bqk": bqk_h,
            "vw": vw_h,
            "vb": vb_h,
            "conf_rep": conf_rep_h,
            "r32": r32_h,
            "m4": m4_h,
            "ones1": ones_h,
            "ident": id_h,
            "ones_h": ones_h.astype(np.float16),
        })
    return in_maps


def kernel(hist_real, hist_imag, current_real, current_imag, confidence,
           qW, qb, kW, kb, vW, vb):
    ppc = PAIRS // N_CORES
    nc = get_nc(ppc)
    in_maps = host_prep(hist_real, hist_imag, current_real, current_imag,
                        confidence, qW, qb, kW, kb, vW, vb, ppc)
    res = run_bass_kernel_spmd(nc, in_maps, list(range(N_CORES))).results
    out = np.concatenate([res[c]["out"] for c in range(N_CORES)], axis=0)
    return out.view(np.complex64)[..., 0].reshape(B, T, D)



# revision 4
# speedup vs baseline: 1.1108x; 1.1108x over previous
"""Trainium2 Bass kernel for ComplexAttention (v2).

Math (per (b,t) pair):
    cur2 = [cur_r, cur_i]                       # [2D]
    Q    = cur2 @ qW + qb                       # [D]
    K_s  = H_s @ kW + kb ; V_s = H_s @ vW + vb  # H = [hist_r, hist_i]  [S, 2D]
    sc_s = (Q . K_s) * scale * conf
    w    = softmax(sc) ; ctx = sum_s w_s V_s
    out  = cur + 0.1 * ctx (complex)

Rewrites (exact):
    Q . K_s = (cur2 @ (qW kW^T) + qb kW^T) . H_s  (+ const-over-s term, dropped)
    ctx = (sum_s w_s H_s) @ vW + vb               (since sum_s w_s = 1)

Per-core structure (512 pairs/core, data-parallel over 8 cores):
  phase A (batch of 128 pairs): Qk = cur2 @ Wqk + bqk  (bf16, overlaps H DMA)
  per quad (16 pairs = 4 groups of 4, one 2MB H tile [128=(j,s), 4, 2D] f32):
    per group: qkr = replicate Qk rows across 32 slots (bf16 row-tiled matmul)
               scores = fused DVE scalar_tensor_tensor:
                   sink = (H * conf*scale) * qkr, accum_out = scores col
    softmax: exp (ACT) -> dn replication matmul (block-ones) -> recip, wn=exp/dn
    per group: wd = m4 * wn[:,g] (gpsimd), hb = wd^T @ H (M=4 matmul),
               hsb = copy hb -> f16 (ACT), gather into hbar (gpsimd DMA)
  per batch: transpose hbar (PE), ctx = hbarT^T @ vW + vb, out = cur + 0.1*ctx
"""

import os
import sys

import numpy as np

os.environ.setdefault("MYCRO_LOCAL_CACHE", "1")

try:
    import concourse.bass as bass
except ImportError:  # pragma: no cover
    sys.path.insert(0, "/opt/trn_rl_repo")
    import concourse.bass as bass

import concourse.mybir as mybir
import concourse.tile as tile
from concourse import bacc
from concourse.bass_utils import run_bass_kernel_spmd

F32 = mybir.dt.float32
F32R = mybir.dt.float32r
F16 = mybir.dt.float16
BF16 = mybir.dt.bfloat16
AX = mybir.AluOpType
AF = mybir.ActivationFunctionType

B, T, S, D = 4, 1024, 32, 512
D2 = 2 * D  # 1024, concat(real, imag) feature dim
E = 2 * D   # 1024, history feature dim
N_CORES = 8
PAIRS = B * T
SCALE = float(D) ** -0.5


def build(ppc: int) -> bass.Bass:
    """Build the per-core SPMD program for `ppc` pairs per core."""
    assert ppc % 128 == 0
    nb = ppc // 128      # batches of 128 pairs
    nq = ppc // 16       # quads of 16 pairs (4 groups)
    ng = ppc // 4        # groups of 4 pairs

    nc = bacc.Bacc("TRN2", target_bir_lowering=False)

    hist_r = nc.declare_dram_parameter("hist_real", [ppc, S, D], F32, isOutput=False)
    hist_i = nc.declare_dram_parameter("hist_imag", [ppc, S, D], F32, isOutput=False)
    cur_r = nc.declare_dram_parameter("cur_r", [ppc, D], F32, isOutput=False)
    cur_i = nc.declare_dram_parameter("cur_i", [ppc, D], F32, isOutput=False)
    cur2t = nc.declare_dram_parameter("cur2t", [D2, ppc], BF16, isOutput=False)
    wqk = nc.declare_dram_parameter("wqk", [D2, E], BF16, isOutput=False)
    bqk = nc.declare_dram_parameter("bqk", [1, E], BF16, isOutput=False)
    vw = nc.declare_dram_parameter("vw", [E, E], F16, isOutput=False)
    vb = nc.declare_dram_parameter("vb", [1, E], F16, isOutput=False)
    conf_rep = nc.declare_dram_parameter("conf_rep", [128, ng], F32, isOutput=False)
    r32 = nc.declare_dram_parameter("r32", [128, 8, 128], BF16, isOutput=False)
    m4 = nc.declare_dram_parameter("m4", [128, 4], F32, isOutput=False)
    bones = nc.declare_dram_parameter("bones", [128, 128], BF16, isOutput=False)
    ones1b = nc.declare_dram_parameter("ones1b", [1, 128], BF16, isOutput=False)
    ident = nc.declare_dram_parameter("ident", [128, 128], F16, isOutput=False)
    ones_h = nc.declare_dram_parameter("ones_h", [1, 128], F16, isOutput=False)
    out = nc.declare_dram_parameter("out", [ppc, D, 2], F32, isOutput=True)

    from contextlib import ExitStack

    with tile.TileContext(nc) as tc, ExitStack() as es:
        ec = es.enter_context
        cpool = ec(tc.tile_pool(name="const", bufs=1))
        wpool = ec(tc.tile_pool(name="bigw", bufs=1))
        curpool = ec(tc.tile_pool(name="cur", bufs=2))
        qkpool = ec(tc.tile_pool(name="qk", bufs=4))
        hpool = ec(tc.tile_pool(name="h", bufs=4))
        sinkpool = ec(tc.tile_pool(name="sink", bufs=2))
        smpool = ec(tc.tile_pool(name="sm", bufs=3))
        wdpool = ec(tc.tile_pool(name="wd", bufs=4))
        hsbpool = ec(tc.tile_pool(name="hsb", bufs=3))
        hbpool = ec(tc.tile_pool(name="hbarb", bufs=2))
        htpool = ec(tc.tile_pool(name="hbarT", bufs=2))
        outpool = ec(tc.tile_pool(name="outp", bufs=2))
        ps_qkr = ec(tc.tile_pool(name="ps_qkr", bufs=2, space="PSUM"))
        ps_hb = ec(tc.tile_pool(name="ps_hb", bufs=2, space="PSUM"))
        ps_misc = ec(tc.tile_pool(name="ps_misc", bufs=2, space="PSUM"))
        del es

        # ---- constants / weights resident in SBUF ----
        m4_t = cpool.tile([128, 4], F32)
        nc.sync.dma_start(out=m4_t[:], in_=m4[:])
        bones_t = cpool.tile([128, 128], BF16)
        nc.sync.dma_start(out=bones_t[:], in_=bones[:])
        ones1b_t = cpool.tile([1, 128], BF16)
        nc.sync.dma_start(out=ones1b_t[:], in_=ones1b[:])
        id_t = cpool.tile([128, 128], F16)
        nc.sync.dma_start(out=id_t[:], in_=ident[:])
        onesh_t = cpool.tile([1, 128], F16)
        nc.sync.dma_start(out=onesh_t[:], in_=ones_h[:])
        bqk_t = cpool.tile([1, E], BF16)
        nc.sync.dma_start(out=bqk_t[:], in_=bqk[:])
        vb_t = cpool.tile([1, E], F16)
        nc.sync.dma_start(out=vb_t[:], in_=vb[:])
        cr_t = cpool.tile([128, ng], F32)
        nc.sync.dma_start(out=cr_t[:], in_=conf_rep[:])
        r32_t = cpool.tile([128, 8, 128], BF16)
        nc.sync.dma_start(out=r32_t[:], in_=r32[:])

        c2t_t = wpool.tile([128, 8, ppc], BF16, tag="c2t")
        nc.sync.dma_start(
            out=c2t_t[:],
            in_=cur2t[:].rearrange("(k p) n -> p k n", p=128),
        )
        wqk_t = wpool.tile([128, 8, E], BF16, tag="wqk")
        nc.sync.dma_start(
            out=wqk_t[:],
            in_=wqk[:].rearrange("(k p) e -> p k e", p=128),
        )
        vw_t = wpool.tile([128, 8, E], F16, tag="vw")
        nc.sync.dma_start(
            out=vw_t[:],
            in_=vw[:].rearrange("(k p) e -> p k e", p=128),
        )

        # ---- phase A: Qk = cur2 @ Wqk + bqk, kept as bf16 per batch ----
        qks = []
        for b in range(nb):
            qk_t = qkpool.tile([128, E], BF16, tag="qk")
            for h in range(2):
                ps = ps_misc.tile([128, 512], F32, tag="misc")
                for k in range(8):
                    nc.tensor.matmul(
                        ps[:],
                        lhsT=c2t_t[:, k, 128 * b : 128 * (b + 1)],
                        rhs=wqk_t[:, k, 512 * h : 512 * (h + 1)],
                        start=(k == 0),
                        stop=False,
                    )
                nc.tensor.matmul(
                    ps[:],
                    lhsT=ones1b_t[:],
                    rhs=bqk_t[:, 512 * h : 512 * (h + 1)],
                    start=False,
                    stop=True,
                )
                nc.scalar.activation(
                    qk_t[:, 512 * h : 512 * (h + 1)], ps[:], AF.Copy
                )
            qks.append(qk_t)

        # ---- phases B (scores/softmax/hbar) + C (ctx/out) per batch ----
        for b in range(nb):
            cur_t = curpool.tile([128, 2, D], F32, tag="cur")
            nc.sync.dma_start(
                out=cur_t[:, 0, :], in_=cur_r[128 * b : 128 * (b + 1), :]
            )
            nc.sync.dma_start(
                out=cur_t[:, 1, :], in_=cur_i[128 * b : 128 * (b + 1), :]
            )
            hbar_b = hbpool.tile([128, E], F16)
            for q in range(8):  # quads of 4 groups = 16 pairs
                sb = q // 2          # 32-pair sub-block of the batch
                h_t = hpool.tile([128, 4, E], F32R, tag="h")
                p0 = 128 * b + 16 * q
                nc.sync.dma_start(
                    out=h_t[:, :, 0:D],
                    in_=hist_r[p0 : p0 + 16].bitcast(F32R).rearrange(
                        "(gl j) s d -> (j s) gl d", j=4
                    ),
                )
                nc.sync.dma_start(
                    out=h_t[:, :, D:E],
                    in_=hist_i[p0 : p0 + 16].bitcast(F32R).rearrange(
                        "(gl j) s d -> (j s) gl d", j=4
                    ),
                )

                scq = smpool.tile([128, 4], F32, tag="scores")
                for gl in range(4):
                    g = 32 * b + 4 * q + gl   # core-local group id
                    gl8 = 4 * (q % 2) + gl    # group index within sub-block
                    qkr = ps_qkr.tile([128, E], F32, tag="qkr")
                    for h in range(2):
                        nc.tensor.matmul(
                            qkr[:, 512 * h : 512 * (h + 1)],
                            lhsT=r32_t[32 * sb : 32 * sb + 32, gl8, :],
                            rhs=qks[b][32 * sb : 32 * sb + 32,
                                       512 * h : 512 * (h + 1)],
                            start=True,
                            stop=True,
                            tile_position=(32 * sb, 0),
                        )
                    sink = sinkpool.tile([128, E], F16, tag="sink")
                    nc.vector.scalar_tensor_tensor(
                        out=sink[:],
                        in0=h_t[:, gl, :].bitcast(F32),
                        scalar=cr_t[:, g : g + 1],
                        in1=qkr[:],
                        op0=AX.mult,
                        op1=AX.mult,
                        accum_out=scq[:, gl : gl + 1],
                    )

                exp4 = smpool.tile([128, 4], BF16, tag="exp")
                nc.scalar.activation(exp4[:], scq[:], AF.Exp)
                dn = ps_misc.tile([128, 4], F32, tag="misc")
                nc.tensor.matmul(dn[:], lhsT=bones_t[:], rhs=exp4[:],
                                 start=True, stop=True)
                invr = smpool.tile([128, 4], F32, tag="invr")
                nc.vector.reciprocal(invr[:], dn[:])
                wn = smpool.tile([128, 4], F32, tag="wn")
                nc.vector.tensor_tensor(out=wn[:], in0=exp4[:], in1=invr[:],
                                        op=AX.mult)

                for gl in range(4):
                    wd = wdpool.tile([128, 4], F32R, tag="wd")
                    nc.gpsimd.tensor_scalar(
                        wd[:], m4_t[:], wn[:, gl : gl + 1], None, AX.mult,
                    )
                    hsb = hsbpool.tile([4, E], F16, tag="hsb")
                    for h in range(2):
                        hb = ps_hb.tile([4, 512], F32, tag="hb")
                        nc.tensor.matmul(
                            hb[:],
                            lhsT=wd[:],
                            rhs=h_t[:, gl, 512 * h : 512 * (h + 1)],
                            start=True,
                            stop=True,
                        )
                        nc.scalar.activation(
                            hsb[:, 512 * h : 512 * (h + 1)], hb[:], AF.Copy
                        )
                    lp = 16 * q + 4 * gl  # batch-local first pair of group
                    nc.gpsimd.dma_start(out=hbar_b[lp : lp + 4, :], in_=hsb[:])

            # ---- phase C: ctx = hbar @ vW + vb ; out = cur + 0.1*ctx ----
            ht0 = htpool.tile([128, 8, 128], F16, tag="hbarT")
            for c in range(8):
                tp = ps_misc.tile([128, 128], F16, tag="misc")
                nc.tensor.transpose(
                    tp[:], hbar_b[:, 128 * c : 128 * (c + 1)], id_t[:]
                )
                nc.scalar.activation(ht0[:, c, :], tp[:], AF.Copy)
            out_t = outpool.tile([128, D, 2], F32)
            for h2 in range(2):
                cps = ps_misc.tile([128, 512], F32, tag="misc")
                for c in range(8):
                    nc.tensor.matmul(
                        cps[:],
                        lhsT=ht0[:, c, :],
                        rhs=vw_t[:, c, 512 * h2 : 512 * (h2 + 1)],
                        start=(c == 0),
                        stop=False,
                    )
                nc.tensor.matmul(
                    cps[:],
                    lhsT=onesh_t[:],
                    rhs=vb_t[:, 512 * h2 : 512 * (h2 + 1)],
                    start=False,
                    stop=True,
                )
                nc.vector.scalar_tensor_tensor(
                    out=out_t[:, :, h2],
                    in0=cps[:],
                    scalar=0.1,
                    in1=cur_t[:, h2, :],
                    op0=AX.mult,
                    op1=AX.add,
                )
            nc.sync.dma_start(
                out=out[:].rearrange("(bb p) d two -> p bb d two", p=128)[:, b],
                in_=out_t[:],
            )

    nc.compile()
    return nc


_CACHE: dict[int, bass.Bass] = {}


def get_nc(ppc: int) -> bass.Bass:
    if ppc not in _CACHE:
        _CACHE[ppc] = build(ppc)
    return _CACHE[ppc]


def _to_bf16(x: np.ndarray) -> np.ndarray:
    """Round f32 -> bf16 bit pattern, returned as uint16 view-compatible array."""
    import ml_dtypes
    return x.astype(ml_dtypes.bfloat16)


def make_const_inputs(ng: int):
    import ml_dtypes
    r32_v = np.zeros((8, 32, 128), np.float32)
    for v in range(8):
        for j in range(4):
            r32_v[v, 4 * v + j, 32 * j : 32 * (j + 1)] = 1.0
    # tiled into each 32-partition block so lhsT base matches rhs base
    r32_h = np.ascontiguousarray(np.tile(r32_v.transpose(1, 0, 2), (4, 1, 1)))
    m4_h = np.zeros((128, 4), np.float32)
    for j in range(4):
        m4_h[32 * j : 32 * (j + 1), j] = 1.0
    bones_h = np.zeros((128, 128), np.float32)
    for j in range(4):
        bones_h[32 * j : 32 * (j + 1), 32 * j : 32 * (j + 1)] = 1.0
    ones_h = np.ones((1, 128), np.float32)
    id_h = np.eye(128, dtype=np.float16)
    return (
        r32_h.astype(ml_dtypes.bfloat16),
        m4_h,
        bones_h.astype(ml_dtypes.bfloat16),
        ones_h.astype(ml_dtypes.bfloat16),
        id_h,
        ones_h.astype(np.float16),
    )


def host_prep(hist_real, hist_imag, current_real, current_imag, confidence,
              qW, qb, kW, kb, vW, vb, ppc):
    """Shared host-side folding + per-core input maps."""
    f = lambda x: np.ascontiguousarray(np.asarray(x, dtype=np.float32))
    hist_real, hist_imag = f(hist_real), f(hist_imag)
    current_real, current_imag = f(current_real), f(current_imag)
    confidence = f(confidence)
    qW, qb, kW, kb, vW, vb = f(qW), f(qb), f(kW), f(kb), f(vW), f(vb)

    n_cores = (B * T) // ppc
    wqk_h = _to_bf16(np.ascontiguousarray(qW @ kW.T))    # [D2, E]
    bqk_h = _to_bf16((qb @ kW.T).reshape(1, E))          # [1, E]
    vw_h = vW.astype(np.float16)
    vb_h = vb.reshape(1, E).astype(np.float16)
    ng = ppc // 4
    r32_h, m4_h, bones_h, ones1b_h, id_h, onesh_h = make_const_inputs(ng)

    hr = hist_real.reshape(B * T, S, D)
    hi = hist_imag.reshape(B * T, S, D)
    cr = current_real.reshape(B * T, D)
    ci = current_imag.reshape(B * T, D)
    cf = confidence.reshape(B * T)

    in_maps = []
    for c in range(n_cores):
        sl = slice(c * ppc, (c + 1) * ppc)
        cur2t_h = _to_bf16(np.ascontiguousarray(
            np.concatenate([cr[sl], ci[sl]], axis=1).T
        ))  # [D2, ppc]
        c4 = cf[sl].reshape(ng, 4).T * SCALE          # [4, ng]
        conf_rep_h = np.ascontiguousarray(np.repeat(c4, 32, axis=0))  # [128, ng]
        in_maps.append({
            "hist_real": hr[sl],
            "hist_imag": hi[sl],
            "cur_r": cr[sl],
            "cur_i": ci[sl],
            "cur2t": cur2t_h,
            "wqk": wqk_h,
            "bqk": bqk_h,
            "vw": vw_h,
            "vb": vb_h,
            "conf_rep": conf_rep_h,
            "r32": r32_h,
            "m4": m4_h,
            "bones": bones_h,
            "ones1b": ones1b_h,
            "ident": id_h,
            "ones_h": onesh_h,
        })
    return in_maps


def kernel(hist_real, hist_imag, current_real, current_imag, confidence,
           qW, qb, kW, kb, vW, vb):
    ppc = PAIRS // N_CORES
    nc = get_nc(ppc)
    in_maps = host_prep(hist_real, hist_imag, current_real, current_imag,
                        confidence, qW, qb, kW, kb, vW, vb, ppc)
    res = run_bass_kernel_spmd(nc, in_maps, list(range(N_CORES))).results
    out = np.concatenate([res[c]["out"] for c in range(N_CORES)], axis=0)
    return out.view(np.complex64)[..., 0].reshape(B, T, D)


# revision 6
# speedup vs baseline: 1.1397x; 1.0260x over previous
"""Trainium2 Bass kernel for ComplexAttention (v2).

Math (per (b,t) pair):
    cur2 = [cur_r, cur_i]                       # [2D]
    Q    = cur2 @ qW + qb                       # [D]
    K_s  = H_s @ kW + kb ; V_s = H_s @ vW + vb  # H = [hist_r, hist_i]  [S, 2D]
    sc_s = (Q . K_s) * scale * conf
    w    = softmax(sc) ; ctx = sum_s w_s V_s
    out  = cur + 0.1 * ctx (complex)

Rewrites (exact):
    Q . K_s = (cur2 @ (qW kW^T) + qb kW^T) . H_s  (+ const-over-s term, dropped)
    ctx = (sum_s w_s H_s) @ vW + vb               (since sum_s w_s = 1)

Per-core structure (512 pairs/core, data-parallel over 8 cores):
  phase A (batch of 128 pairs): Qk = cur2 @ Wqk + bqk  (bf16, overlaps H DMA)
  per quad (16 pairs = 4 groups of 4, one 2MB H tile [128=(j,s), 4, 2D] f32):
    per group: qkr = replicate Qk rows across 32 slots (bf16 row-tiled matmul)
               scores = fused DVE scalar_tensor_tensor:
                   sink = (H * conf*scale) * qkr, accum_out = scores col
    softmax: exp (ACT) -> dn replication matmul (block-ones) -> recip, wn=exp/dn
    per group: wd = m4 * wn[:,g] (gpsimd), hb = wd^T @ H (M=4 matmul),
               hsb = copy hb -> f16 (ACT), gather into hbar (gpsimd DMA)
  per batch: transpose hbar (PE), ctx = hbarT^T @ vW + vb, out = cur + 0.1*ctx
"""

import os
import sys

import numpy as np

os.environ.setdefault("MYCRO_LOCAL_CACHE", "1")

try:
    import concourse.bass as bass
except ImportError:  # pragma: no cover
    sys.path.insert(0, "/opt/trn_rl_repo")
    import concourse.bass as bass

import concourse.mybir as mybir
import concourse.tile as tile
from concourse import bacc
from concourse.bass_utils import run_bass_kernel_spmd

F32 = mybir.dt.float32
F32R = mybir.dt.float32r
F16 = mybir.dt.float16
BF16 = mybir.dt.bfloat16
AX = mybir.AluOpType
AF = mybir.ActivationFunctionType

B, T, S, D = 4, 1024, 32, 512
D2 = 2 * D  # 1024, concat(real, imag) feature dim
E = 2 * D   # 1024, history feature dim
N_CORES = 8
PAIRS = B * T
SCALE = float(D) ** -0.5


def build(ppc: int) -> bass.Bass:
    """Build the per-core SPMD program for `ppc` pairs per core."""
    assert ppc % 128 == 0
    nb = ppc // 128      # batches of 128 pairs
    nq = ppc // 16       # quads of 16 pairs (4 groups)
    ng = ppc // 4        # groups of 4 pairs

    nc = bacc.Bacc("TRN2", target_bir_lowering=False)

    hist_r = nc.declare_dram_parameter("hist_real", [ppc, S, D], F32, isOutput=False)
    hist_i = nc.declare_dram_parameter("hist_imag", [ppc, S, D], F32, isOutput=False)
    cur_r = nc.declare_dram_parameter("cur_r", [ppc, D], F32, isOutput=False)
    cur_i = nc.declare_dram_parameter("cur_i", [ppc, D], F32, isOutput=False)
    cur2t = nc.declare_dram_parameter("cur2t", [D2, ppc], BF16, isOutput=False)
    wqk = nc.declare_dram_parameter("wqk", [D2, E], BF16, isOutput=False)
    bqk = nc.declare_dram_parameter("bqk", [1, E], BF16, isOutput=False)
    vw = nc.declare_dram_parameter("vw", [E, E], F16, isOutput=False)
    vb = nc.declare_dram_parameter("vb", [1, E], F16, isOutput=False)
    conf_rep = nc.declare_dram_parameter("conf_rep", [128, ng], F32, isOutput=False)
    r32 = nc.declare_dram_parameter("r32", [128, 8, 128], BF16, isOutput=False)
    m4 = nc.declare_dram_parameter("m4", [128, 4], F32, isOutput=False)
    ones1b = nc.declare_dram_parameter("ones1b", [1, 128], BF16, isOutput=False)
    ident = nc.declare_dram_parameter("ident", [128, 128], F16, isOutput=False)
    ones_h = nc.declare_dram_parameter("ones_h", [1, 128], F16, isOutput=False)
    out = nc.declare_dram_parameter("out", [ppc, D, 2], F32, isOutput=True)

    from contextlib import ExitStack

    with tile.TileContext(nc) as tc, ExitStack() as es:
        ec = es.enter_context
        cpool = ec(tc.tile_pool(name="const", bufs=1))
        wpool = ec(tc.tile_pool(name="bigw", bufs=1))
        curpool = ec(tc.tile_pool(name="cur", bufs=2))
        qkpool = ec(tc.tile_pool(name="qk", bufs=4))
        hpool = ec(tc.tile_pool(name="h", bufs=4))
        sinkpool = ec(tc.tile_pool(name="sink", bufs=2))
        smpool = ec(tc.tile_pool(name="sm", bufs=3))
        wdpool = ec(tc.tile_pool(name="wd", bufs=4))
        hsbpool = ec(tc.tile_pool(name="hsb", bufs=3))
        hbpool = ec(tc.tile_pool(name="hbarb", bufs=2))
        htpool = ec(tc.tile_pool(name="hbarT", bufs=2))
        outpool = ec(tc.tile_pool(name="outp", bufs=2))
        ps_qkr = ec(tc.tile_pool(name="ps_qkr", bufs=2, space="PSUM"))
        ps_hb = ec(tc.tile_pool(name="ps_hb", bufs=2, space="PSUM"))
        ps_misc = ec(tc.tile_pool(name="ps_misc", bufs=2, space="PSUM"))
        del es

        # ---- constants / weights resident in SBUF ----
        m4_t = cpool.tile([128, 4], F32)
        nc.sync.dma_start(out=m4_t[:], in_=m4[:])
        ones1b_t = cpool.tile([1, 128], BF16)
        nc.sync.dma_start(out=ones1b_t[:], in_=ones1b[:])
        id_t = cpool.tile([128, 128], F16)
        nc.sync.dma_start(out=id_t[:], in_=ident[:])
        onesh_t = cpool.tile([1, 128], F16)
        nc.sync.dma_start(out=onesh_t[:], in_=ones_h[:])
        bqk_t = cpool.tile([1, E], BF16)
        nc.sync.dma_start(out=bqk_t[:], in_=bqk[:])
        vb_t = cpool.tile([1, E], F16)
        nc.sync.dma_start(out=vb_t[:], in_=vb[:])
        cr_t = cpool.tile([128, ng], F32)
        nc.sync.dma_start(out=cr_t[:], in_=conf_rep[:])
        r32_t = cpool.tile([128, 8, 128], BF16)
        nc.sync.dma_start(out=r32_t[:], in_=r32[:])

        c2t_t = wpool.tile([128, 8, ppc], BF16, tag="c2t")
        nc.sync.dma_start(
            out=c2t_t[:],
            in_=cur2t[:].rearrange("(k p) n -> p k n", p=128),
        )
        wqk_t = wpool.tile([128, 8, E], BF16, tag="wqk")
        nc.sync.dma_start(
            out=wqk_t[:],
            in_=wqk[:].rearrange("(k p) e -> p k e", p=128),
        )
        vw_t = wpool.tile([128, 8, E], F16, tag="vw")
        nc.sync.dma_start(
            out=vw_t[:],
            in_=vw[:].rearrange("(k p) e -> p k e", p=128),
        )

        # ---- phase A: Qk = cur2 @ Wqk + bqk, kept as bf16 per batch ----
        qks = []
        for b in range(nb):
            qk_t = qkpool.tile([128, E], BF16, tag="qk")
            for h in range(2):
                ps = ps_misc.tile([128, 512], F32, tag="misc")
                for k in range(8):
                    nc.tensor.matmul(
                        ps[:],
                        lhsT=c2t_t[:, k, 128 * b : 128 * (b + 1)],
                        rhs=wqk_t[:, k, 512 * h : 512 * (h + 1)],
                        start=(k == 0),
                        stop=False,
                    )
                nc.tensor.matmul(
                    ps[:],
                    lhsT=ones1b_t[:],
                    rhs=bqk_t[:, 512 * h : 512 * (h + 1)],
                    start=False,
                    stop=True,
                )
                nc.scalar.activation(
                    qk_t[:, 512 * h : 512 * (h + 1)], ps[:], AF.Copy
                )
            qks.append(qk_t)

        # ---- phases B + C: software-pipelined over quads ----
        # iter i: scores(i) [PE repl + DVE STT], softhb(i-1) [ACT/GPS/PE],
        #         tail(i) [exp/dn/inv/wd].  C(b) emitted one iter after b ends.
        nq_total = nq
        h_tiles = {}
        scq_t = {}
        exp_t = {}
        inv_t = {}
        wd_t = {}
        hbar_t = {}
        cur_tiles = {}
        pend_c = None

        def emit_load(i):
            if i >= nq_total:
                return
            b, q = divmod(i, 8)
            if q == 0:
                cur_t = curpool.tile([128, 2, D], F32, tag="cur")
                nc.sync.dma_start(
                    out=cur_t[:, 0, :], in_=cur_r[128 * b : 128 * (b + 1), :]
                )
                nc.sync.dma_start(
                    out=cur_t[:, 1, :], in_=cur_i[128 * b : 128 * (b + 1), :]
                )
                cur_tiles[b] = cur_t
            h_t = hpool.tile([128, 4, E], F32R, tag="h")
            p0 = 16 * i
            nc.sync.dma_start(
                out=h_t[:, :, 0:D],
                in_=hist_r[p0 : p0 + 16].bitcast(F32R).rearrange(
                    "(gl j) s d -> (j s) gl d", j=4
                ),
            )
            nc.sync.dma_start(
                out=h_t[:, :, D:E],
                in_=hist_i[p0 : p0 + 16].bitcast(F32R).rearrange(
                    "(gl j) s d -> (j s) gl d", j=4
                ),
            )
            h_tiles[i] = h_t

        def emit_scores(i):
            b, q = divmod(i, 8)
            sb = q // 2
            h_t = h_tiles[i]
            scq = smpool.tile([128, 4], F32, tag="scores")
            scq_t[i] = scq
            for gl in range(4):
                g = 32 * b + 4 * q + gl
                gl8 = 4 * (q % 2) + gl
                qkr = ps_qkr.tile([128, E], F32, tag="qkr")
                for h in range(2):
                    nc.tensor.matmul(
                        qkr[:, 512 * h : 512 * (h + 1)],
                        lhsT=r32_t[32 * sb : 32 * sb + 32, gl8, :],
                        rhs=qks[b][32 * sb : 32 * sb + 32,
                                   512 * h : 512 * (h + 1)],
                        start=True,
                        stop=True,
                        tile_position=(32 * sb, 0),
                    )
                sink = sinkpool.tile([128, E], F16, tag="sink")
                nc.vector.scalar_tensor_tensor(
                    out=sink[:],
                    in0=h_t[:, gl, :].bitcast(F32),
                    scalar=cr_t[:, g : g + 1],
                    in1=qkr[:],
                    op0=AX.mult,
                    op1=AX.mult,
                    accum_out=scq[:, gl : gl + 1],
                )

        def emit_softhb(i):
            b, q = divmod(i, 8)
            if q == 0:
                hbar_new = hbpool.tile([128, E], F16, tag="hbar")
                hbar_t[b] = hbar_new
            hbar_b = hbar_t[b]
            h_t = h_tiles.pop(i)
            inv4 = inv_t.pop(i)
            for gl in range(4):
                wd = wd_t.pop((i, gl))
                hsb = hsbpool.tile([4, E], F16, tag="hsb")
                for h in range(2):
                    hb = ps_hb.tile([4, 512], F32, tag="hb")
                    nc.tensor.matmul(
                        hb[:],
                        lhsT=wd[:],
                        rhs=h_t[:, gl, 512 * h : 512 * (h + 1)],
                        start=True,
                        stop=True,
                    )
                    nc.scalar.activation(
                        hsb[:, 512 * h : 512 * (h + 1)], hb[:], AF.Copy,
                        scale=inv4[:, gl : gl + 1],
                    )
                lp = 16 * q + 4 * gl
                nc.gpsimd.dma_start(out=hbar_b[lp : lp + 4, :], in_=hsb[:])

        def emit_tail(i):
            scq = scq_t.pop(i)
            exp4 = smpool.tile([128, 4], F32, tag="exp")
            nc.scalar.activation(exp4[:], scq[:], AF.Exp)
            dn = ps_misc.tile([4, 4], F32, tag="misc")
            nc.tensor.matmul(dn[:], lhsT=m4_t[:], rhs=exp4[:],
                             start=True, stop=True)
            inv4 = smpool.tile([4, 4], F32, tag="invr")
            nc.vector.reciprocal(inv4[:], dn[:])
            inv_t[i] = inv4
            for gl in range(4):
                wd = wdpool.tile([128, 4], F32R, tag="wd")
                nc.gpsimd.tensor_scalar(
                    wd[:], m4_t[:], exp4[:, gl : gl + 1], None, AX.mult,
                )
                wd_t[(i, gl)] = wd

        def emit_c(b):
            hbar_b = hbar_t.pop(b)
            cur_t = cur_tiles.pop(b)
            ht0 = htpool.tile([128, 8, 128], F16, tag="hbarT")
            for c in range(8):
                tp = ps_misc.tile([128, 128], F16, tag="misc")
                nc.tensor.transpose(
                    tp[:], hbar_b[:, 128 * c : 128 * (c + 1)], id_t[:]
                )
                nc.scalar.activation(ht0[:, c, :], tp[:], AF.Copy)
            out_t = outpool.tile([128, D, 2], F32)
            for h2 in range(2):
                cps = ps_misc.tile([128, 512], F32, tag="misc")
                for c in range(8):
                    nc.tensor.matmul(
                        cps[:],
                        lhsT=ht0[:, c, :],
                        rhs=vw_t[:, c, 512 * h2 : 512 * (h2 + 1)],
                        start=(c == 0),
                        stop=False,
                    )
                nc.tensor.matmul(
                    cps[:],
                    lhsT=onesh_t[:],
                    rhs=vb_t[:, 512 * h2 : 512 * (h2 + 1)],
                    start=False,
                    stop=True,
                )
                nc.vector.scalar_tensor_tensor(
                    out=out_t[:, :, h2],
                    in0=cps[:],
                    scalar=0.1,
                    in1=cur_t[:, h2, :],
                    op0=AX.mult,
                    op1=AX.add,
                )
            nc.sync.dma_start(
                out=out[:].rearrange("(bb p) d two -> p bb d two", p=128)[:, b],
                in_=out_t[:],
            )

        for i in range(3):
            emit_load(i)
        for i in range(nq_total + 1):
            emit_load(i + 3)
            if i < nq_total:
                emit_scores(i)
            if pend_c is not None:
                emit_c(pend_c)
                pend_c = None
            if i >= 1:
                emit_softhb(i - 1)
                if i % 8 == 0:
                    pend_c = i // 8 - 1
            if i < nq_total:
                emit_tail(i)
        emit_c(nb - 1)

    nc.compile()
    return nc


_CACHE: dict[int, bass.Bass] = {}


def get_nc(ppc: int) -> bass.Bass:
    if ppc not in _CACHE:
        _CACHE[ppc] = build(ppc)
    return _CACHE[ppc]


def _to_bf16(x: np.ndarray) -> np.ndarray:
    """Round f32 -> bf16 bit pattern, returned as uint16 view-compatible array."""
    import ml_dtypes
    return x.astype(ml_dtypes.bfloat16)


def make_const_inputs(ng: int):
    import ml_dtypes
    r32_v = np.zeros((8, 32, 128), np.float32)
    for v in range(8):
        for j in range(4):
            r32_v[v, 4 * v + j, 32 * j : 32 * (j + 1)] = 1.0
    # tiled into each 32-partition block so lhsT base matches rhs base
    r32_h = np.ascontiguousarray(np.tile(r32_v.transpose(1, 0, 2), (4, 1, 1)))
    m4_h = np.zeros((128, 4), np.float32)
    for j in range(4):
        m4_h[32 * j : 32 * (j + 1), j] = 1.0
    ones_h = np.ones((1, 128), np.float32)
    id_h = np.eye(128, dtype=np.float16)
    return (
        r32_h.astype(ml_dtypes.bfloat16),
        m4_h,
        ones_h.astype(ml_dtypes.bfloat16),
        id_h,
        ones_h.astype(np.float16),
    )


def host_prep(hist_real, hist_imag, current_real, current_imag, confidence,
              qW, qb, kW, kb, vW, vb, ppc):
    """Shared host-side folding + per-core input maps."""
    f = lambda x: np.ascontiguousarray(np.asarray(x, dtype=np.float32))
    hist_real, hist_imag = f(hist_real), f(hist_imag)
    current_real, current_imag = f(current_real), f(current_imag)
    confidence = f(confidence)
    qW, qb, kW, kb, vW, vb = f(qW), f(qb), f(kW), f(kb), f(vW), f(vb)

    n_cores = (B * T) // ppc
    wqk_h = _to_bf16(np.ascontiguousarray(qW @ kW.T))    # [D2, E]
    bqk_h = _to_bf16((qb @ kW.T).reshape(1, E))          # [1, E]
    vw_h = vW.astype(np.float16)
    vb_h = vb.reshape(1, E).astype(np.float16)
    ng = ppc // 4
    r32_h, m4_h, ones1b_h, id_h, onesh_h = make_const_inputs(ng)

    hr = hist_real.reshape(B * T, S, D)
    hi = hist_imag.reshape(B * T, S, D)
    cr = current_real.reshape(B * T, D)
    ci = current_imag.reshape(B * T, D)
    cf = confidence.reshape(B * T)

    in_maps = []
    for c in range(n_cores):
        sl = slice(c * ppc, (c + 1) * ppc)
        cur2t_h = _to_bf16(np.ascontiguousarray(
            np.concatenate([cr[sl], ci[sl]], axis=1).T
        ))  # [D2, ppc]
        c4 = cf[sl].reshape(ng, 4).T * SCALE          # [4, ng]
        conf_rep_h = np.ascontiguousarray(np.repeat(c4, 32, axis=0))  # [128, ng]
        in_maps.append({
            "hist_real": hr[sl],
            "hist_imag": hi[sl],
            "cur_r": cr[sl],
            "cur_i": ci[sl],
            "cur2t": cur2t_h,
            "wqk": wqk_h,
            "bqk": bqk_h,
            "vw": vw_h,
            "vb": vb_h,
            "conf_rep": conf_rep_h,
            "r32": r32_h,
            "m4": m4_h,
            "ones1b": ones1b_h,
            "ident": id_h,
            "ones_h": onesh_h,
        })
    return in_maps


def kernel(hist_real, hist_imag, current_real, current_imag, confidence,
           qW, qb, kW, kb, vW, vb):
    ppc = PAIRS // N_CORES
    nc = get_nc(ppc)
    in_maps = host_prep(hist_real, hist_imag, current_real, current_imag,
                        confidence, qW, qb, kW, kb, vW, vb, ppc)
    res = run_bass_kernel_spmd(nc, in_maps, list(range(N_CORES))).results
    out = np.concatenate([res[c]["out"] for c in range(N_CORES)], axis=0)
    return out.view(np.complex64)[..., 0].reshape(B, T, D)


# revision 7
# speedup vs baseline: 1.2110x; 1.0625x over previous
"""Trainium2 Bass kernel for ComplexAttention (v2).

Math (per (b,t) pair):
    cur2 = [cur_r, cur_i]                       # [2D]
    Q    = cur2 @ qW + qb                       # [D]
    K_s  = H_s @ kW + kb ; V_s = H_s @ vW + vb  # H = [hist_r, hist_i]  [S, 2D]
    sc_s = (Q . K_s) * scale * conf
    w    = softmax(sc) ; ctx = sum_s w_s V_s
    out  = cur + 0.1 * ctx (complex)

Rewrites (exact):
    Q . K_s = (cur2 @ (qW kW^T) + qb kW^T) . H_s  (+ const-over-s term, dropped)
    ctx = (sum_s w_s H_s) @ vW + vb               (since sum_s w_s = 1)

Per-core structure (512 pairs/core, data-parallel over 8 cores):
  phase A (batch of 128 pairs): Qk = cur2 @ Wqk + bqk  (bf16, overlaps H DMA)
  per quad (16 pairs = 4 groups of 4, one 2MB H tile [128=(j,s), 4, 2D] f32):
    per group: qkr = replicate Qk rows across 32 slots (bf16 row-tiled matmul)
               scores = fused DVE scalar_tensor_tensor:
                   sink = (H * conf*scale) * qkr, accum_out = scores col
    softmax: exp (ACT) -> dn replication matmul (block-ones) -> recip, wn=exp/dn
    per group: wd = m4 * wn[:,g] (gpsimd), hb = wd^T @ H (M=4 matmul),
               hsb = copy hb -> f16 (ACT), gather into hbar (gpsimd DMA)
  per batch: transpose hbar (PE), ctx = hbarT^T @ vW + vb, out = cur + 0.1*ctx
"""

import os
import sys

import numpy as np

os.environ.setdefault("MYCRO_LOCAL_CACHE", "1")

try:
    import concourse.bass as bass
except ImportError:  # pragma: no cover
    sys.path.insert(0, "/opt/trn_rl_repo")
    import concourse.bass as bass

import concourse.mybir as mybir
import concourse.tile as tile
from concourse import bacc
from concourse.bass_utils import run_bass_kernel_spmd

F32 = mybir.dt.float32
F32R = mybir.dt.float32r
F16 = mybir.dt.float16
BF16 = mybir.dt.bfloat16
AX = mybir.AluOpType
AF = mybir.ActivationFunctionType

B, T, S, D = 4, 1024, 32, 512
D2 = 2 * D  # 1024, concat(real, imag) feature dim
E = 2 * D   # 1024, history feature dim
N_CORES = 8
PAIRS = B * T
SCALE = float(D) ** -0.5


def build(ppc: int) -> bass.Bass:
    """Build the per-core SPMD program for `ppc` pairs per core."""
    assert ppc % 128 == 0
    nb = ppc // 128      # batches of 128 pairs
    nq = ppc // 16       # quads of 16 pairs (4 groups)
    ng = ppc // 4        # groups of 4 pairs

    nc = bacc.Bacc("TRN2", target_bir_lowering=False)

    hist_r = nc.declare_dram_parameter("hist_real", [ppc, S, D], F32, isOutput=False)
    hist_i = nc.declare_dram_parameter("hist_imag", [ppc, S, D], F32, isOutput=False)
    cur_r = nc.declare_dram_parameter("cur_r", [ppc, D], F32, isOutput=False)
    cur_i = nc.declare_dram_parameter("cur_i", [ppc, D], F32, isOutput=False)
    cur2t = nc.declare_dram_parameter("cur2t", [D2, ppc], BF16, isOutput=False)
    wqk = nc.declare_dram_parameter("wqk", [D2, E], BF16, isOutput=False)
    bqk = nc.declare_dram_parameter("bqk", [1, E], BF16, isOutput=False)
    vw = nc.declare_dram_parameter("vw", [E, E], F16, isOutput=False)
    conf_rep = nc.declare_dram_parameter("conf_rep", [128, ng], F32, isOutput=False)
    mask32 = nc.declare_dram_parameter("mask32", [128, 32, 128], BF16, isOutput=False)
    m4 = nc.declare_dram_parameter("m4", [128, 4], F32, isOutput=False)
    ones1b = nc.declare_dram_parameter("ones1b", [1, 128], BF16, isOutput=False)
    ident = nc.declare_dram_parameter("ident", [128, 128], F16, isOutput=False)
    out = nc.declare_dram_parameter("out", [ppc, D, 2], F32, isOutput=True)

    from contextlib import ExitStack

    with tile.TileContext(nc) as tc, ExitStack() as es:
        ec = es.enter_context
        cpool = ec(tc.tile_pool(name="const", bufs=1))
        wpool = ec(tc.tile_pool(name="bigw", bufs=1))
        curpool = ec(tc.tile_pool(name="cur", bufs=2))
        qkpool = ec(tc.tile_pool(name="qk", bufs=4))
        hpool = ec(tc.tile_pool(name="h", bufs=4))
        sinkpool = ec(tc.tile_pool(name="sink", bufs=2))
        smpool = ec(tc.tile_pool(name="sm", bufs=3))
        wdpool = ec(tc.tile_pool(name="wd", bufs=4))
        hsbpool = ec(tc.tile_pool(name="hsb", bufs=3))
        hbpool = ec(tc.tile_pool(name="hbarb", bufs=2))
        htpool = ec(tc.tile_pool(name="hbarT", bufs=2))
        outpool = ec(tc.tile_pool(name="outp", bufs=2))
        ps_qkr = ec(tc.tile_pool(name="ps_qkr", bufs=2, space="PSUM"))
        ps_hb = ec(tc.tile_pool(name="ps_hb", bufs=2, space="PSUM"))
        ps_misc = ec(tc.tile_pool(name="ps_misc", bufs=2, space="PSUM"))
        del es

        # ---- constants / weights resident in SBUF ----
        m4_t = cpool.tile([128, 4], F32)
        nc.sync.dma_start(out=m4_t[:], in_=m4[:])
        ones1b_t = cpool.tile([1, 128], BF16)
        nc.sync.dma_start(out=ones1b_t[:], in_=ones1b[:])
        id_t = cpool.tile([128, 128], F16)
        nc.sync.dma_start(out=id_t[:], in_=ident[:])
        bqk_t = cpool.tile([1, E], BF16)
        nc.sync.dma_start(out=bqk_t[:], in_=bqk[:])
        cr_t = cpool.tile([128, ng], F32)
        nc.sync.dma_start(out=cr_t[:], in_=conf_rep[:])
        mask_t = cpool.tile([128, 32, 128], BF16)
        nc.sync.dma_start(out=mask_t[:], in_=mask32[:])

        c2t_t = wpool.tile([128, 8, ppc], BF16, tag="c2t")
        nc.sync.dma_start(
            out=c2t_t[:],
            in_=cur2t[:].rearrange("(k p) n -> p k n", p=128),
        )
        wqk_t = wpool.tile([128, 8, E], BF16, tag="wqk")
        nc.sync.dma_start(
            out=wqk_t[:],
            in_=wqk[:].rearrange("(k p) e -> p k e", p=128),
        )
        vw_t = wpool.tile([128, 8, E], F16, tag="vw")
        nc.sync.dma_start(
            out=vw_t[:],
            in_=vw[:].rearrange("(k p) e -> p k e", p=128),
        )

        # ---- phase A: Qk = cur2 @ Wqk + bqk, kept as bf16 per batch ----
        qks = []
        for b in range(nb):
            qk_t = qkpool.tile([128, E], BF16, tag="qk")
            for h in range(2):
                ps = ps_misc.tile([128, 512], F32, tag="misc")
                for k in range(8):
                    nc.tensor.matmul(
                        ps[:],
                        lhsT=c2t_t[:, k, 128 * b : 128 * (b + 1)],
                        rhs=wqk_t[:, k, 512 * h : 512 * (h + 1)],
                        start=(k == 0),
                        stop=False,
                    )
                nc.tensor.matmul(
                    ps[:],
                    lhsT=ones1b_t[:],
                    rhs=bqk_t[:, 512 * h : 512 * (h + 1)],
                    start=False,
                    stop=True,
                )
                nc.scalar.activation(
                    qk_t[:, 512 * h : 512 * (h + 1)], ps[:], AF.Copy
                )
            qks.append(qk_t)

        # ---- phases B + C: software-pipelined over quads ----
        # iter i: scores(i) [PE repl + DVE STT], softhb(i-1) [ACT/GPS/PE],
        #         tail(i) [exp/dn/inv/wd].  C(b) emitted one iter after b ends.
        nq_total = nq
        h_tiles = {}
        scq_t = {}
        exp_t = {}
        inv_t = {}
        wd_t = {}
        hbar_t = {}
        cur_tiles = {}
        pend_c = None

        def emit_load(i):
            if i >= nq_total:
                return
            b, q = divmod(i, 8)
            if q == 0:
                cur_t = curpool.tile([128, 2, D], F32, tag="cur")
                nc.sync.dma_start(
                    out=cur_t[:, 0, :], in_=cur_r[128 * b : 128 * (b + 1), :]
                )
                nc.sync.dma_start(
                    out=cur_t[:, 1, :], in_=cur_i[128 * b : 128 * (b + 1), :]
                )
                cur_tiles[b] = cur_t
            h_t = hpool.tile([128, 4, E], F32R, tag="h")
            p0 = 16 * i
            nc.sync.dma_start(
                out=h_t[:, :, 0:D],
                in_=hist_r[p0 : p0 + 16].bitcast(F32R).rearrange(
                    "(gl j) s d -> (j s) gl d", j=4
                ),
            )
            nc.sync.dma_start(
                out=h_t[:, :, D:E],
                in_=hist_i[p0 : p0 + 16].bitcast(F32R).rearrange(
                    "(gl j) s d -> (j s) gl d", j=4
                ),
            )
            h_tiles[i] = h_t

        def emit_scores(i):
            b, q = divmod(i, 8)
            h_t = h_tiles[i]
            scq = smpool.tile([128, 4], F32, tag="scores")
            scq_t[i] = scq
            for gl in range(4):
                g = 32 * b + 4 * q + gl
                lg = 4 * q + gl           # batch-local group id
                qkr = ps_qkr.tile([128, E], F32, tag="qkr")
                for h in range(2):
                    nc.tensor.matmul(
                        qkr[:, 512 * h : 512 * (h + 1)],
                        lhsT=mask_t[:, lg, :],
                        rhs=qks[b][:, 512 * h : 512 * (h + 1)],
                        start=True,
                        stop=True,
                    )
                sink = sinkpool.tile([128, E], F16, tag="sink")
                nc.vector.scalar_tensor_tensor(
                    out=sink[:],
                    in0=h_t[:, gl, :].bitcast(F32),
                    scalar=cr_t[:, g : g + 1],
                    in1=qkr[:],
                    op0=AX.mult,
                    op1=AX.mult,
                    accum_out=scq[:, gl : gl + 1],
                )

        def emit_softhb(i):
            b, q = divmod(i, 8)
            if q == 0:
                hbar_new = hbpool.tile([128, E], F16, tag="hbar")
                hbar_t[b] = hbar_new
            hbar_b = hbar_t[b]
            h_t = h_tiles.pop(i)
            inv4 = inv_t.pop(i)
            for gl in range(4):
                wd = wd_t.pop((i, gl))
                hsb = hsbpool.tile([4, E], F16, tag="hsb")
                for h in range(2):
                    hb = ps_hb.tile([4, 512], F32, tag="hb")
                    nc.tensor.matmul(
                        hb[:],
                        lhsT=wd[:],
                        rhs=h_t[:, gl, 512 * h : 512 * (h + 1)],
                        start=True,
                        stop=True,
                    )
                    nc.scalar.activation(
                        hsb[:, 512 * h : 512 * (h + 1)], hb[:], AF.Copy,
                        scale=inv4[:, gl : gl + 1],
                    )
                lp = 16 * q + 4 * gl
                nc.gpsimd.dma_start(out=hbar_b[lp : lp + 4, :], in_=hsb[:])

        def emit_tail(i):
            scq = scq_t.pop(i)
            exp4 = smpool.tile([128, 4], F32, tag="exp")
            nc.scalar.activation(exp4[:], scq[:], AF.Exp)
            dn = ps_misc.tile([4, 4], F32, tag="misc")
            nc.tensor.matmul(dn[:], lhsT=m4_t[:], rhs=exp4[:],
                             start=True, stop=True)
            inv4 = smpool.tile([4, 4], F32, tag="invr")
            nc.vector.reciprocal(inv4[:], dn[:])
            inv_t[i] = inv4
            for gl in range(4):
                wd = wdpool.tile([128, 4], F32R, tag="wd")
                nc.gpsimd.tensor_scalar(
                    wd[:], m4_t[:], exp4[:, gl : gl + 1], None, AX.mult,
                )
                wd_t[(i, gl)] = wd

        def emit_c(b):
            hbar_b = hbar_t.pop(b)
            cur_t = cur_tiles.pop(b)
            ht0 = htpool.tile([128, 8, 128], F16, tag="hbarT")
            for c in range(8):
                tp = ps_misc.tile([128, 128], F16, tag="misc")
                nc.tensor.transpose(
                    tp[:], hbar_b[:, 128 * c : 128 * (c + 1)], id_t[:]
                )
                nc.scalar.activation(ht0[:, c, :], tp[:], AF.Copy)
            out_t = outpool.tile([128, D, 2], F32)
            for h2 in range(2):
                cps = ps_misc.tile([128, 512], F32, tag="misc")
                for c in range(8):
                    nc.tensor.matmul(
                        cps[:],
                        lhsT=ht0[:, c, :],
                        rhs=vw_t[:, c, 512 * h2 : 512 * (h2 + 1)],
                        start=(c == 0),
                        stop=(c == 7),
                    )
                nc.vector.scalar_tensor_tensor(
                    out=out_t[:, :, h2],
                    in0=cps[:],
                    scalar=0.1,
                    in1=cur_t[:, h2, :],
                    op0=AX.mult,
                    op1=AX.add,
                )
            nc.sync.dma_start(
                out=out[:].rearrange("(bb p) d two -> p bb d two", p=128)[:, b],
                in_=out_t[:],
            )

        for i in range(3):
            emit_load(i)
        for i in range(nq_total + 1):
            emit_load(i + 3)
            if i < nq_total:
                emit_scores(i)
            if pend_c is not None:
                emit_c(pend_c)
                pend_c = None
            if i >= 1:
                emit_softhb(i - 1)
                if i % 8 == 0:
                    pend_c = i // 8 - 1
            if i < nq_total:
                emit_tail(i)
        emit_c(nb - 1)

    nc.compile()
    return nc


_CACHE: dict[int, bass.Bass] = {}


def get_nc(ppc: int) -> bass.Bass:
    if ppc not in _CACHE:
        _CACHE[ppc] = build(ppc)
    return _CACHE[ppc]


def _to_bf16(x: np.ndarray) -> np.ndarray:
    """Round f32 -> bf16 bit pattern, returned as uint16 view-compatible array."""
    import ml_dtypes
    return x.astype(ml_dtypes.bfloat16)


def make_const_inputs(ng: int):
    import ml_dtypes
    mask_v = np.zeros((128, 32, 128), np.float32)
    for lg in range(32):
        for j in range(4):
            mask_v[4 * lg + j, lg, 32 * j : 32 * (j + 1)] = 1.0
    m4_h = np.zeros((128, 4), np.float32)
    for j in range(4):
        m4_h[32 * j : 32 * (j + 1), j] = 1.0
    ones_h = np.ones((1, 128), np.float32)
    id_h = np.eye(128, dtype=np.float16)
    return (
        mask_v.astype(ml_dtypes.bfloat16),
        m4_h,
        ones_h.astype(ml_dtypes.bfloat16),
        id_h,
    )


def host_prep(hist_real, hist_imag, current_real, current_imag, confidence,
              qW, qb, kW, kb, vW, vb, ppc):
    """Shared host-side folding + per-core input maps."""
    f = lambda x: np.ascontiguousarray(np.asarray(x, dtype=np.float32))
    hist_real, hist_imag = f(hist_real), f(hist_imag)
    current_real, current_imag = f(current_real), f(current_imag)
    confidence = f(confidence)
    qW, qb, kW, kb, vW, vb = f(qW), f(qb), f(kW), f(kb), f(vW), f(vb)

    n_cores = (B * T) // ppc
    wqk_h = _to_bf16(np.ascontiguousarray(qW @ kW.T))    # [D2, E]
    bqk_h = _to_bf16((qb @ kW.T).reshape(1, E))          # [1, E]
    vw_h = vW.astype(np.float16)
    ng = ppc // 4
    mask_h, m4_h, ones1b_h, id_h = make_const_inputs(ng)

    hr = hist_real.reshape(B * T, S, D)
    hi = hist_imag.reshape(B * T, S, D)
    cr = current_real.reshape(B * T, D)
    ci = current_imag.reshape(B * T, D)
    cf = confidence.reshape(B * T)

    in_maps = []
    for c in range(n_cores):
        sl = slice(c * ppc, (c + 1) * ppc)
        cur2t_h = _to_bf16(np.ascontiguousarray(
            np.concatenate([cr[sl], ci[sl]], axis=1).T
        ))  # [D2, ppc]
        c4 = cf[sl].reshape(ng, 4).T * SCALE          # [4, ng]
        conf_rep_h = np.ascontiguousarray(np.repeat(c4, 32, axis=0))  # [128, ng]
        in_maps.append({
            "hist_real": hr[sl],
            "hist_imag": hi[sl],
            "cur_r": cr[sl],
            "cur_i": ci[sl],
            "cur2t": cur2t_h,
            "wqk": wqk_h,
            "bqk": bqk_h,
            "vw": vw_h,
            "conf_rep": conf_rep_h,
            "mask32": mask_h,
            "m4": m4_h,
            "ones1b": ones1b_h,
            "ident": id_h,
        })
    return in_maps


def kernel(hist_real, hist_imag, current_real, current_imag, confidence,
           qW, qb, kW, kb, vW, vb):
    ppc = PAIRS // N_CORES
    nc = get_nc(ppc)
    in_maps = host_prep(hist_real, hist_imag, current_real, current_imag,
                        confidence, qW, qb, kW, kb, vW, vb, ppc)
    res = run_bass_kernel_spmd(nc, in_maps, list(range(N_CORES))).results
    out = np.concatenate([res[c]["out"] for c in range(N_CORES)], axis=0)
    out = out.view(np.complex64)[..., 0].reshape(B, T, D)
    vb_f = np.asarray(vb, dtype=np.float32).reshape(E)
    out = out + 0.1 * (vb_f[:D] + 1j * vb_f[D:]).astype(np.complex64)
    return out
